# revision 13
# baseline (speedup 1.0000x reference)
"""3-layer GAT (8 heads x 64 ch) + global mean pool + FC + log_softmax on 8 Trainium2 cores.

Sharding: nodes (and their incoming edges) are partitioned across the 8 cores;
GAT weights are replicated; per layer each core computes h=x@W for its node
shard, the [h | a_src] rows are AllGathered into a replicated table, and each
core gathers source rows for its edges with indirect DMA. Segment softmax and
the weighted scatter-add are both expressed as one-hot matmuls on the PE
(contract over a 128-edge chunk), accumulating numerator and denominator in
PSUM per 128-node dst tile. Graph mean-pool partials are AllReduced at the end.
"""

import numpy as np

import concourse.bass as bass
import concourse.mybir as mybir
import concourse.tile as tile
from concourse import bacc
from concourse.bass_utils import run_bass_kernel_spmd
from concourse.masks import make_identity

# problem constants (hardcoded per contract)
N, E, F_IN, H, C, G, NCLS = 50000, 400000, 128, 8, 64, 64, 10
HC = H * C  # 512
NEG = 0.2
EPS = 1e-16

NCORES = 8
P = 128
NSH = N // NCORES          # 6250 nodes per core
NT = (NSH + P - 1) // P    # 49 dst tiles per core
NSHP = NT * P              # 6272 padded rows per core
NFULL = NCORES * NSHP      # 50176 rows in the gathered table
ROWW = HC + H              # 520: h | a_src
PAD = 999.0                # one-hot miss marker for padded edge slots

F32 = mybir.dt.float32
I32 = mybir.dt.int32

M_GATHER = 1  # HW indirect DMA honors only one index per partition per op


def _preprocess_edges(edge_index):
    """Assign each edge (incl. self loops) to the core owning its dst, group by
    128-node dst tile, pad each tile's edge list to a multiple of 128.

    Returns (CTS, esrcT, edstT): CTS[t] = chunk count of tile t (shared by all
    cores); esrcT[k] int32 [128, TOT] gather row ids; edstT[k] f32 [128, TOT]
    dst slot within tile (or PAD)."""
    src = np.concatenate([edge_index[0], np.arange(N, dtype=np.int64)])
    dst = np.concatenate([edge_index[1], np.arange(N, dtype=np.int64)])
    core = dst // NSH
    dloc = dst - core * NSH
    tile_of = dloc // P
    slot = dloc - tile_of * P
    gid = (src // NSH) * NSHP + (src % NSH)  # row in padded gathered table

    counts = np.zeros((NCORES, NT), np.int64)
    np.add.at(counts, (core, tile_of), 1)
    CTS = np.maximum((counts.max(axis=0) + P - 1) // P, 1).astype(np.int64)
    TOT = int(CTS.sum())
    chunk_base = np.zeros(NT, np.int64)
    chunk_base[1:] = np.cumsum(CTS)[:-1]

    esrcT = np.zeros((NCORES, P, TOT), np.int32)
    edstT = np.full((NCORES, P, TOT), PAD, np.float32)
    for k in range(NCORES):
        m = core == k
        t_k, s_k, g_k = tile_of[m], slot[m], gid[m]
        order = np.argsort(t_k, kind="stable")
        t_k, s_k, g_k = t_k[order], s_k[order], g_k[order]
        # within-tile running index
        start = np.zeros(NT + 1, np.int64)
        np.add.at(start[1:], t_k, 1)
        start = np.cumsum(start)
        j = np.arange(len(t_k)) - start[t_k]
        ch = chunk_base[t_k] + j // P
        pp = j % P
        esrcT[k, pp, ch] = g_k.astype(np.int32)
        edstT[k, pp, ch] = s_k.astype(np.float32)
    return CTS, esrcT, edstT


def _ext_weights(W, a_s, a_d):
    """[K, 512+16] = [W | W@A_s | W@A_d] so h, a_src, a_dst come from one matmul."""
    K = W.shape[0]
    Wr = W.reshape(K, H, C)
    ws = np.einsum("fhc,hc->fh", Wr, a_s)
    wd = np.einsum("fhc,hc->fh", Wr, a_d)
    Wx = np.concatenate([W, ws, wd], axis=1).astype(np.float32)  # [K, 528]
    nk = K // P
    return np.ascontiguousarray(Wx.reshape(nk, P, HC + 2 * H).transpose(1, 0, 2))


def _build_nc(CTS, debug=False, mode='full'):
    TOT = int(CTS.sum())
    nc = bacc.Bacc("TRN2", target_bir_lowering=False, debug=False,
                   num_devices=NCORES)

    x_ext = nc.dram_tensor("x0", [NSHP, F_IN], F32, kind="ExternalInput")
    esrc_ext = nc.dram_tensor("esrc", [P, TOT], I32, kind="ExternalInput")
    edst_ext = nc.dram_tensor("edst", [P, TOT], F32, kind="ExternalInput")
    w1_ext = nc.dram_tensor("w1", [P, 1, HC + 2 * H], F32, kind="ExternalInput")
    w2_ext = nc.dram_tensor("w2", [P, 4, HC + 2 * H], F32, kind="ExternalInput")
    w3_ext = nc.dram_tensor("w3", [P, 4, HC + 2 * H], F32, kind="ExternalInput")
    b1_ext = nc.dram_tensor("b1r", [P, HC], F32, kind="ExternalInput")
    b2_ext = nc.dram_tensor("b2r", [P, HC], F32, kind="ExternalInput")
    b3_ext = nc.dram_tensor("b3r", [P, C], F32, kind="ExternalInput")
    pool_ext = nc.dram_tensor("poolidx", [P, NT], F32, kind="ExternalInput")
    invc_ext = nc.dram_tensor("invcnt", [G, 1], F32, kind="ExternalInput")
    fcw_ext = nc.dram_tensor("fcw", [C, NCLS], F32, kind="ExternalInput")
    fcb_ext = nc.dram_tensor("fcbr", [G, NCLS], F32, kind="ExternalInput")
    out_ext = nc.dram_tensor("out", [G, NCLS], F32, kind="ExternalOutput")
    if mode == "noA":
        hxref_ext = nc.dram_tensor("hxref", [NSHP, ROWW], F32,
                                   kind="ExternalInput")
    if debug:
        dbg_hx = nc.dram_tensor("dbg_hx", [NSHP, ROWW], F32, kind="ExternalOutput")
        dbg_x2 = nc.dram_tensor("dbg_x2", [NSHP, HC], F32, kind="ExternalOutput")
        dbg_g = nc.dram_tensor("dbg_g", [P, M_GATHER, ROWW], F32, kind="ExternalOutput")
        dbg_den = nc.dram_tensor("dbg_den", [P, H], F32, kind="ExternalOutput")
        dbg_hxf = nc.dram_tensor("dbg_hxf", [4 * P, ROWW], F32, kind="ExternalOutput")
        dbg_es = nc.dram_tensor("dbg_es", [P, 4], I32, kind="ExternalOutput")

    rg = [list(range(NCORES))]

    with tile.TileContext(nc) as tc:
        with (
            tc.tile_pool(name="const", bufs=1) as cpool,
            tc.tile_pool(name="work", bufs=3) as wpool,
            tc.tile_pool(name="gat", bufs=4) as gpool,
            tc.tile_pool(name="ps", bufs=1, space="PSUM") as pspool,
            tc.tile_pool(name="acc", bufs=1, space="PSUM") as accpool,
            tc.tile_pool(name="dram", bufs=1, space="DRAM") as dpool,
        ):
            # ---- constants ----
            iota_i = cpool.tile([P, P], I32)
            nc.gpsimd.iota(iota_i[:], pattern=[[1, P]], base=0, channel_multiplier=0)
            iota_f = cpool.tile([P, P], F32)
            nc.vector.tensor_copy(iota_f[:], iota_i[:])
            ident = cpool.tile([P, P], F32)
            make_identity(nc, ident[:])

            w1_s = cpool.tile([P, 1, HC + 2 * H], F32)
            nc.sync.dma_start(out=w1_s[:], in_=w1_ext[:])
            w2_s = cpool.tile([P, 4, HC + 2 * H], F32)
            nc.sync.dma_start(out=w2_s[:], in_=w2_ext[:])
            w3_s = cpool.tile([P, 4, HC + 2 * H], F32)
            nc.sync.dma_start(out=w3_s[:], in_=w3_ext[:])
            b1_s = cpool.tile([P, HC], F32)
            nc.sync.dma_start(out=b1_s[:], in_=b1_ext[:])
            b2_s = cpool.tile([P, HC], F32)
            nc.sync.dma_start(out=b2_s[:], in_=b2_ext[:])
            b3_s = cpool.tile([P, C], F32)
            nc.sync.dma_start(out=b3_s[:], in_=b3_ext[:])
            pool_s = cpool.tile([P, NT], F32)
            nc.sync.dma_start(out=pool_s[:], in_=pool_ext[:])
            invc_s = cpool.tile([G, 1], F32)
            nc.sync.dma_start(out=invc_s[:], in_=invc_ext[:])
            fcw_s = cpool.tile([C, NCLS], F32)
            nc.sync.dma_start(out=fcw_s[:], in_=fcw_ext[:])
            fcb_s = cpool.tile([G, NCLS], F32)
            nc.sync.dma_start(out=fcb_s[:], in_=fcb_ext[:])
            adst_all = cpool.tile([P, NT * H], F32)

            # ---- DRAM buffers ----
            hx_local = dpool.tile([NSHP, ROWW], F32)
            hx_fulls = [
                dpool.tile([NFULL, ROWW], F32, addr_space="Shared",
                           name=f"hx_full{i}")
                for i in range(3)
            ]
            xb = dpool.tile([NSHP, HC], F32)
            xc = dpool.tile([NSHP, HC], F32)
            pool_in = dpool.tile([G, C], F32)
            pool_out = dpool.tile([G, C], F32, addr_space="Shared")

            pool_ps = accpool.tile([G, C], F32)

            for layer in range(3):
                K = F_IN if layer == 0 else HC
                nk = K // P
                xsrc = (x_ext, xb, xc)[layer]
                hx_full = hx_fulls[layer]
                w_s = (w1_s, w2_s, w3_s)[layer]
                b_s = (b1_s, b2_s, b3_s)[layer]

                # ---- stage A: h | a_src | a_dst = x @ W_ext ----
                if mode == "noA":
                    if layer == 0:
                        nc.vector.memset(adst_all[:], 0.0)
                    for t in range(NT):
                        st = wpool.tile([P, ROWW], F32, tag="st")
                        nc.sync.dma_start(out=st[:],
                                          in_=hxref_ext[t * P:(t + 1) * P, :])
                        nc.sync.dma_start(out=hx_local[t * P:(t + 1) * P, :],
                                          in_=st[:])
                for t in range(NT if mode != "noA" else 0):
                    xt = wpool.tile([P, K], F32, tag="xt")
                    nc.sync.dma_start(out=xt[:], in_=xsrc[t * P:(t + 1) * P, :])
                    h_ps = pspool.tile([P, HC], F32, tag="big", bufs=2)
                    a_ps = pspool.tile([P, 2 * H], F32, tag="small", bufs=3)
                    for j in range(nk):
                        xT_ps = pspool.tile([P, P], F32, tag="trans", bufs=2)
                        nc.tensor.transpose(out=xT_ps[:], in_=xt[:, j * P:(j + 1) * P],
                                            identity=ident[:])
                        xT = wpool.tile([P, P], F32, tag="xT")
                        nc.scalar.copy(xT[:], xT_ps[:])
                        nc.tensor.matmul(out=h_ps[:], lhsT=xT[:], rhs=w_s[:, j, 0:HC],
                                         start=(j == 0), stop=(j == nk - 1))
                        nc.tensor.matmul(out=a_ps[:], lhsT=xT[:],
                                         rhs=w_s[:, j, HC:HC + 2 * H],
                                         start=(j == 0), stop=(j == nk - 1))
                    hx_t = wpool.tile([P, ROWW], F32, tag="hx_t")
                    nc.vector.tensor_copy(hx_t[:, 0:HC // 2], h_ps[:, 0:HC // 2])
                    nc.scalar.copy(hx_t[:, HC // 2:HC], h_ps[:, HC // 2:HC])
                    nc.vector.tensor_copy(hx_t[:, HC:ROWW], a_ps[:, 0:H])
                    nc.vector.tensor_copy(adst_all[:, t * H:(t + 1) * H],
                                          a_ps[:, H:2 * H])
                    nc.sync.dma_start(out=hx_local[t * P:(t + 1) * P, :], in_=hx_t[:])

                if debug and layer == 0:
                    nc.sync.dma_start(out=dbg_hx[:], in_=hx_local[:])
                # ---- stage B: replicate [h | a_src] across cores ----
                nc.gpsimd.collective_compute(
                    "AllGather", mybir.AluOpType.bypass, replica_groups=rg,
                    ins=[hx_local[:]], outs=[hx_full[:]],
                )

                if debug and layer == 0:
                    for bi, base in enumerate([0, NSHP, 2 * NSHP, 7 * NSHP]):
                        hxf_t = wpool.tile([P, ROWW], F32, tag="hxf_t")
                        nc.sync.dma_start(out=hxf_t[:],
                                          in_=hx_full[base:base + P, :])
                        nc.sync.dma_start(out=dbg_hxf[bi * P:(bi + 1) * P, :],
                                          in_=hxf_t[:])
                # ---- stage C: edge aggregation per dst tile ----
                ch0 = 0
                for t in range(NT):
                    Ct = int(CTS[t])
                    es_t = wpool.tile([P, Ct], I32, tag="es_t")
                    nc.sync.dma_start(out=es_t[:], in_=esrc_ext[:, ch0:ch0 + Ct])
                    ed_t = wpool.tile([P, Ct], F32, tag="ed_t")
                    nc.sync.dma_start(out=ed_t[:], in_=edst_ext[:, ch0:ch0 + Ct])
                    if debug and layer == 0 and t == 0:
                        nc.sync.dma_start(out=dbg_es[:], in_=es_t[:, 0:4])
                    num_ps = pspool.tile([P, HC], F32, tag="big", bufs=2)
                    den_ps = pspool.tile([P, H], F32, tag="small", bufs=3)
                    a_d = adst_all[:, t * H:(t + 1) * H]

                    for c in range(Ct):
                        if True:
                            gt = gpool.tile([P, ROWW], F32, tag="gt")
                            nc.gpsimd.indirect_dma_start(
                                out=gt[:], out_offset=None,
                                in_=hx_full[:],
                                in_offset=bass.IndirectOffsetOnAxis(
                                    ap=es_t[:, c:c + 1], axis=0),
                            )
                            if debug and layer == 0 and t == 0 and c == 0:
                                nc.sync.dma_start(out=dbg_g[:, 0, :], in_=gt[:])
                            g = gt[:]
                            oh = gpool.tile([P, P], F32, tag="oh")
                            nc.vector.tensor_tensor(
                                out=oh[:], in0=ed_t[:, c:c + 1].to_broadcast([P, P]),
                                in1=iota_f[:], op=mybir.AluOpType.is_equal)
                            ohT_ps = pspool.tile([P, P], F32, tag="trans", bufs=2)
                            nc.tensor.transpose(out=ohT_ps[:], in_=oh[:],
                                                identity=ident[:])
                            ohT = gpool.tile([P, P], F32, tag="ohT")
                            nc.scalar.copy(ohT[:], ohT_ps[:])
                            adpe_ps = pspool.tile([P, H], F32, tag="small", bufs=3)
                            nc.tensor.matmul(out=adpe_ps[:], lhsT=ohT[:], rhs=a_d,
                                             start=True, stop=True)
                            logit = gpool.tile([P, H], F32, tag="logit")
                            nc.vector.tensor_add(logit[:], g[:, HC:ROWW], adpe_ps[:])
                            lr = gpool.tile([P, H], F32, tag="lr")
                            nc.vector.tensor_scalar_mul(lr[:], logit[:], NEG)
                            nc.vector.tensor_tensor(out=lr[:], in0=lr[:], in1=logit[:],
                                                    op=mybir.AluOpType.max)
                            ex = gpool.tile([P, H], F32, tag="ex")
                            nc.scalar.activation(ex[:], lr[:],
                                                 mybir.ActivationFunctionType.Exp)
                            msg = gpool.tile([P, HC], F32, tag="msg")
                            for h in range(H):
                                dst_sl = msg[:, h * C:(h + 1) * C]
                                src_sl = g[:, h * C:(h + 1) * C]
                                if h % 2 == 0:
                                    nc.vector.tensor_scalar_mul(dst_sl, src_sl,
                                                                ex[:, h:h + 1])
                                else:
                                    nc.scalar.mul(dst_sl, src_sl, ex[:, h:h + 1])
                            nc.tensor.matmul(out=num_ps[:], lhsT=oh[:], rhs=msg[:],
                                             start=(c == 0), stop=(c == Ct - 1))
                            nc.tensor.matmul(out=den_ps[:], lhsT=oh[:], rhs=ex[:],
                                             start=(c == 0), stop=(c == Ct - 1))

                    den = wpool.tile([P, H], F32, tag="den")
                    nc.vector.tensor_scalar_add(den[:], den_ps[:], EPS)
                    if debug and layer == 0 and t == 0:
                        nc.sync.dma_start(out=dbg_den[:], in_=den[:])
                    rec = wpool.tile([P, H], F32, tag="rec")
                    nc.vector.reciprocal(rec[:], den[:])
                    if layer < 2:
                        ot = wpool.tile([P, HC], F32, tag="ot")
                        for h in range(H):
                            dst_sl = ot[:, h * C:(h + 1) * C]
                            src_sl = num_ps[:, h * C:(h + 1) * C]
                            if h % 2 == 0:
                                nc.vector.tensor_scalar_mul(dst_sl, src_sl,
                                                            rec[:, h:h + 1])
                            else:
                                nc.scalar.mul(dst_sl, src_sl, rec[:, h:h + 1])
                        ot2 = wpool.tile([P, HC], F32, tag="ot2")
                        nc.vector.tensor_add(ot2[:], ot[:], b_s[:])
                        ot3 = wpool.tile([P, HC], F32, tag="ot3")
                        nc.scalar.activation(ot3[:], ot2[:],
                                             mybir.ActivationFunctionType.Relu)
                        xdst = (xb, xc)[layer]
                        nc.sync.dma_start(out=xdst[t * P:(t + 1) * P, :], in_=ot3[:])
                        if debug and layer == 0 and t == NT - 1:
                            nc.sync.dma_start(out=dbg_x2[:], in_=xb[:])
                    else:
                        hm = wpool.tile([P, C], F32, tag="hm")
                        nc.vector.tensor_scalar_mul(hm[:], num_ps[:, 0:C],
                                                    rec[:, 0:1])
                        for h in range(1, H):
                            hmt = wpool.tile([P, C], F32, tag="hmt")
                            if h % 2 == 0:
                                nc.vector.tensor_scalar_mul(
                                    hmt[:], num_ps[:, h * C:(h + 1) * C],
                                    rec[:, h:h + 1])
                            else:
                                nc.scalar.mul(hmt[:], num_ps[:, h * C:(h + 1) * C],
                                              rec[:, h:h + 1])
                            nc.vector.tensor_add(hm[:], hm[:], hmt[:])
                        hs = wpool.tile([P, C], F32, tag="hs")
                        nc.vector.tensor_scalar_mul(hs[:], hm[:], 1.0 / H)
                        nc.vector.tensor_add(hs[:], hs[:], b3_s[:])
                        h3 = wpool.tile([P, C], F32, tag="h3")
                        nc.scalar.activation(h3[:], hs[:],
                                             mybir.ActivationFunctionType.Relu)
                        poh = wpool.tile([P, G], F32, tag="poh")
                        nc.vector.tensor_tensor(
                            out=poh[:], in0=pool_s[:, t:t + 1].to_broadcast([P, G]),
                            in1=iota_f[:, 0:G], op=mybir.AluOpType.is_equal)
                        nc.tensor.matmul(out=pool_ps[:], lhsT=poh[:], rhs=h3[:],
                                         start=(t == 0), stop=(t == NT - 1))
                    ch0 += Ct

            # ---- pool AllReduce + FC + log_softmax ----
            psb = wpool.tile([G, C], F32)
            nc.vector.tensor_copy(psb[:], pool_ps[:])
            nc.sync.dma_start(out=pool_in[:], in_=psb[:])
            nc.gpsimd.collective_compute(
                "AllReduce", mybir.AluOpType.add, replica_groups=rg,
                ins=[pool_in[:]], outs=[pool_out[:]],
            )
            pld = wpool.tile([G, C], F32)
            nc.sync.dma_start(out=pld[:], in_=pool_out[:])
            nc.vector.tensor_scalar_mul(pld[:], pld[:], invc_s[:, 0:1])
            pT_ps = pspool.tile([G, C], F32, tag="small", bufs=3)
            nc.tensor.transpose(out=pT_ps[:], in_=pld[:], identity=ident[0:G, 0:C])
            pT = wpool.tile([C, G], F32)
            nc.scalar.copy(pT[:], pT_ps[:])
            z_ps = pspool.tile([G, NCLS], F32, tag="small", bufs=3)
            nc.tensor.matmul(out=z_ps[:], lhsT=pT[:], rhs=fcw_s[:],
                             start=True, stop=True)
            z = wpool.tile([G, NCLS], F32)
            nc.vector.tensor_add(z[:], z_ps[:], fcb_s[:])
            zm = wpool.tile([G, 1], F32)
            nc.vector.tensor_reduce(zm[:], z[:], axis=mybir.AxisListType.X,
                                    op=mybir.AluOpType.max)
            zs = wpool.tile([G, NCLS], F32)
            nc.vector.tensor_scalar_sub(zs[:], z[:], zm[:, 0:1])
            ze = wpool.tile([G, NCLS], F32)
            nc.scalar.activation(ze[:], zs[:], mybir.ActivationFunctionType.Exp)
            zsum = wpool.tile([G, 1], F32)
            nc.vector.tensor_reduce(zsum[:], ze[:], axis=mybir.AxisListType.X,
                                    op=mybir.AluOpType.add)
            zl = wpool.tile([G, 1], F32)
            nc.scalar.activation(zl[:], zsum[:], mybir.ActivationFunctionType.Ln)
            zo = wpool.tile([G, NCLS], F32)
            nc.vector.tensor_scalar_sub(zo[:], zs[:], zl[:, 0:1])
            nc.sync.dma_start(out=out_ext[:], in_=zo[:])

    nc.compile()
    return nc


def _prepare_inputs(x, edge_index, batch, W1, a1s, a1d, b1, W2, a2s, a2d, b2,
                    W3, a3s, a3d, b3, fcw, fcb):
    CTS, esrcT, edstT = _preprocess_edges(np.asarray(edge_index, np.int64))
    x = np.asarray(x, np.float32)
    batch = np.asarray(batch, np.int64)

    shared = {
        "w1": _ext_weights(np.asarray(W1, np.float32), np.asarray(a1s, np.float32),
                           np.asarray(a1d, np.float32)),
        "w2": _ext_weights(np.asarray(W2, np.float32), np.asarray(a2s, np.float32),
                           np.asarray(a2d, np.float32)),
        "w3": _ext_weights(np.asarray(W3, np.float32), np.asarray(a3s, np.float32),
                           np.asarray(a3d, np.float32)),
        "b1r": np.tile(np.asarray(b1, np.float32)[None, :], (P, 1)),
        "b2r": np.tile(np.asarray(b2, np.float32)[None, :], (P, 1)),
        "b3r": np.tile(np.asarray(b3, np.float32)[None, :], (P, 1)),
        "fcw": np.asarray(fcw, np.float32),
        "fcbr": np.tile(np.asarray(fcb, np.float32)[None, :], (G, 1)),
        "invcnt": (1.0 / np.maximum(
            np.bincount(batch, minlength=G), 1.0)).astype(np.float32)[:, None],
    }

    in_maps = []
    for k in range(NCORES):
        xk = np.zeros((NSHP, F_IN), np.float32)
        xk[:NSH] = x[k * NSH:(k + 1) * NSH]
        pidx = np.full((NSHP,), PAD, np.float32)
        pidx[:NSH] = batch[k * NSH:(k + 1) * NSH]
        poolidx = np.ascontiguousarray(pidx.reshape(NT, P).T)  # [P, NT]
        in_maps.append({
            "x0": xk,
            "esrc": np.ascontiguousarray(esrcT[k]),
            "edst": np.ascontiguousarray(edstT[k]),
            "poolidx": poolidx,
            **shared,
        })
    return CTS, in_maps


_CACHE = {}


def _get_nc(CTS):
    key = tuple(int(c) for c in CTS)
    if key not in _CACHE:
        _CACHE[key] = _build_nc(CTS)
    return _CACHE[key]


def kernel(x, edge_index, batch, W1, a1s, a1d, b1, W2, a2s, a2d, b2,
           W3, a3s, a3d, b3, fcw, fcb, _trace=False, _results=None):
    CTS, in_maps = _prepare_inputs(x, edge_index, batch, W1, a1s, a1d, b1,
                                   W2, a2s, a2d, b2, W3, a3s, a3d, b3, fcw, fcb)
    nc = _get_nc(CTS)
    res = run_bass_kernel_spmd(nc, in_maps, core_ids=list(range(NCORES)),
                               trace=_trace)
    if _results is not None:
        _results.append(res)
    return res.results[0]["out"]


# revision 16
# speedup vs baseline: 1.3646x; 1.3646x over previous
"""3-layer GAT (8 heads x 64 ch) + global mean pool + FC + log_softmax on 8 Trainium2 cores.

Sharding: nodes (and their incoming edges) are partitioned across the 8 cores;
GAT weights are replicated; per layer each core computes h=x@W for its node
shard, the [h | a_src] rows are AllGathered into a replicated table, and each
core gathers source rows for its edges with indirect DMA (one row per
partition per op — the HW limit).

Edge aggregation per 128-node dst tile is hybrid:
- "round" columns: round r holds each node's r-th incoming edge in that
  node's own partition, so dst == partition. a_dst adds directly, padding is
  a -1e30 bias before exp, and the weighted scatter-add is an
  identity-stationary matmul accumulating into PSUM.
- "overflow" columns: edges beyond the per-tile round cap go through a
  one-hot matmul path (is_equal against iota, PE transpose for the per-edge
  a_dst gather).
Numerator and softmax denominator accumulate in the same PSUM group; one
divide per tile. The next layer's x@W (stage A) is fused into the per-tile
finalize so PE work hides under the gather-DMA-bound phase. Graph mean-pool
partials are AllReduced at the end.
"""

import numpy as np

import concourse.bass as bass
import concourse.mybir as mybir
import concourse.tile as tile
from concourse import bacc
from concourse.bass_utils import run_bass_kernel_spmd
from concourse.masks import make_identity

# problem constants (hardcoded per contract)
N, E, F_IN, H, C, G, NCLS = 50000, 400000, 128, 8, 64, 64, 10
HC = H * C  # 512
NEG = 0.2
EPS = 1e-16

NCORES = 8
P = 128
NSH = N // NCORES          # 6250 nodes per core
NT = (NSH + P - 1) // P    # 49 dst tiles per core
NSHP = NT * P              # 6272 padded rows per core
NFULL = NCORES * NSHP      # 50176 rows in the gathered table
ROWW = HC + H              # 520: h | a_src
WEXT = HC + 2 * H          # 528: W | Wa_src | Wa_dst
PAD = 999.0                # one-hot miss marker for padded overflow slots
MASKNEG = -1.0e30          # pre-exp bias masking padded round slots

F32 = mybir.dt.float32
I32 = mybir.dt.int32


def _preprocess_edges(edge_index):
    """Assign edges (incl. self loops) to the dst-owning core; build per-tile
    round columns (node's r-th edge in its own partition) plus overflow
    one-hot chunk columns.

    Returns (plan, esrcT, edstT):
      plan: list of (R_t, C_t) per tile — shared by all cores.
      esrcT[k] int32 [P, TOTCOL]: gather row ids (pad -> 0).
      edstT[k] f32 [P, TOTCOL]: round cols -> 0.0 real / MASKNEG pad;
                                chunk cols -> dst slot or PAD."""
    src = np.concatenate([edge_index[0], np.arange(N, dtype=np.int64)])
    dst = np.concatenate([edge_index[1], np.arange(N, dtype=np.int64)])
    core = dst // NSH
    dloc = dst - core * NSH
    tile_of = dloc // P
    slot = dloc - tile_of * P
    gid = (src // NSH) * NSHP + (src % NSH)

    deg = np.zeros((NCORES, NT, P), np.int64)
    np.add.at(deg, (core, tile_of, slot), 1)
    maxdeg_t = deg.max(axis=(0, 2))  # [NT]

    best = None
    for rcap in range(3, 32):
        R_t = np.minimum(maxdeg_t, rcap)
        ovf = np.maximum(deg - rcap, 0).sum(axis=2)        # [NCORES, NT]
        C_t = np.ceil(ovf / P).astype(np.int64).max(axis=0)
        # gathers dominate; one-hot chunk columns carry ~25% extra compute
        tot = float(R_t.sum() + C_t.sum()) + 0.25 * float(C_t.sum())
        if best is None or tot < best[0]:
            best = (tot, R_t.copy(), C_t.copy(), rcap)
    _, R_ts, C_ts, rcap = best
    plan = [(int(R_ts[t]), int(C_ts[t])) for t in range(NT)]
    colbase = np.zeros(NT, np.int64)
    colbase[1:] = np.cumsum(R_ts + C_ts)[:-1]
    TOTCOL = int((R_ts + C_ts).sum())

    esrcT = np.zeros((NCORES, P, TOTCOL), np.int32)
    edstT = np.empty((NCORES, P, TOTCOL), np.float32)
    for k in range(NCORES):
        # default fill: rounds masked, chunks PAD
        for t in range(NT):
            b = colbase[t]
            edstT[k, :, b:b + R_ts[t]] = MASKNEG
            edstT[k, :, b + R_ts[t]:b + R_ts[t] + C_ts[t]] = PAD
        m = core == k
        t_k, s_k, g_k = tile_of[m], slot[m], gid[m]
        order = np.argsort(t_k * P + s_k, kind="stable")
        t_k, s_k, g_k = t_k[order], s_k[order], g_k[order]
        node = t_k * P + s_k
        start = np.zeros(NT * P + 1, np.int64)
        np.add.at(start[1:], node, 1)
        start = np.cumsum(start)
        j = np.arange(len(node)) - start[node]  # rank within node
        rmax = R_ts[t_k]
        isr = j < rmax
        # round entries
        esrcT[k, s_k[isr], colbase[t_k[isr]] + j[isr]] = g_k[isr].astype(np.int32)
        edstT[k, s_k[isr], colbase[t_k[isr]] + j[isr]] = 0.0
        # overflow entries: sequential position within each tile
        to, so, go = t_k[~isr], s_k[~isr], g_k[~isr]
        oorder = np.argsort(to, kind="stable")
        to, so, go = to[oorder], so[oorder], go[oorder]
        ostart = np.zeros(NT + 1, np.int64)
        np.add.at(ostart[1:], to, 1)
        ostart = np.cumsum(ostart)
        q = np.arange(len(to)) - ostart[to]
        col = colbase[to] + R_ts[to] + q // P
        row = q % P
        esrcT[k, row, col] = go.astype(np.int32)
        edstT[k, row, col] = so.astype(np.float32)
    return plan, esrcT, edstT


def _ext_weights(W, a_s, a_d):
    """[K, 528] = [W | W@A_s | W@A_d] so h, a_src, a_dst come from one matmul."""
    K = W.shape[0]
    Wr = W.reshape(K, H, C)
    ws = np.einsum("fhc,hc->fh", Wr, a_s)
    wd = np.einsum("fhc,hc->fh", Wr, a_d)
    Wx = np.concatenate([W, ws, wd], axis=1).astype(np.float32)
    nk = K // P
    return np.ascontiguousarray(Wx.reshape(nk, P, WEXT).transpose(1, 0, 2))


def _build_nc(plan):
    TOTCOL = sum(r + c for r, c in plan)
    nc = bacc.Bacc("TRN2", target_bir_lowering=False, debug=False,
                   num_devices=NCORES)

    x_ext = nc.dram_tensor("x0", [NSHP, F_IN], F32, kind="ExternalInput")
    esrc_ext = nc.dram_tensor("esrc", [P, TOTCOL], I32, kind="ExternalInput")
    edst_ext = nc.dram_tensor("edst", [P, TOTCOL], F32, kind="ExternalInput")
    w1_ext = nc.dram_tensor("w1", [P, 1, WEXT], F32, kind="ExternalInput")
    w2_ext = nc.dram_tensor("w2", [P, 4, WEXT], F32, kind="ExternalInput")
    w3_ext = nc.dram_tensor("w3", [P, 4, WEXT], F32, kind="ExternalInput")
    b1_ext = nc.dram_tensor("b1r", [P, HC], F32, kind="ExternalInput")
    b2_ext = nc.dram_tensor("b2r", [P, HC], F32, kind="ExternalInput")
    b3_ext = nc.dram_tensor("b3r", [P, C], F32, kind="ExternalInput")
    pool_ext = nc.dram_tensor("poolidx", [P, NT], F32, kind="ExternalInput")
    invc_ext = nc.dram_tensor("invcnt", [G, 1], F32, kind="ExternalInput")
    fcw_ext = nc.dram_tensor("fcw", [C, NCLS], F32, kind="ExternalInput")
    fcb_ext = nc.dram_tensor("fcbr", [G, NCLS], F32, kind="ExternalInput")
    out_ext = nc.dram_tensor("out", [G, NCLS], F32, kind="ExternalOutput")

    rg = [list(range(NCORES))]

    with tile.TileContext(nc) as tc:
        with (
            tc.tile_pool(name="const", bufs=1) as cpool,
            tc.tile_pool(name="work", bufs=3) as wpool,
            tc.tile_pool(name="gat", bufs=6) as gpool,
            tc.tile_pool(name="ps", bufs=1, space="PSUM") as pspool,
            tc.tile_pool(name="dram", bufs=1, space="DRAM") as dpool,
        ):
            # ---- constants ----
            iota_i = cpool.tile([P, P], I32)
            nc.gpsimd.iota(iota_i[:], pattern=[[1, P]], base=0, channel_multiplier=0)
            iota_f = cpool.tile([P, P], F32)
            nc.vector.tensor_copy(iota_f[:], iota_i[:])
            ident = cpool.tile([P, P], F32)
            make_identity(nc, ident[:])

            w1_s = cpool.tile([P, 1, WEXT], F32)
            nc.sync.dma_start(out=w1_s[:], in_=w1_ext[:])
            w2_s = cpool.tile([P, 4, WEXT], F32)
            nc.sync.dma_start(out=w2_s[:], in_=w2_ext[:])
            w3_s = cpool.tile([P, 4, WEXT], F32)
            nc.sync.dma_start(out=w3_s[:], in_=w3_ext[:])
            b1_s = cpool.tile([P, HC], F32)
            nc.sync.dma_start(out=b1_s[:], in_=b1_ext[:])
            b2_s = cpool.tile([P, HC], F32)
            nc.sync.dma_start(out=b2_s[:], in_=b2_ext[:])
            b3_s = cpool.tile([P, C], F32)
            nc.sync.dma_start(out=b3_s[:], in_=b3_ext[:])
            pool_s = cpool.tile([P, NT], F32)
            nc.sync.dma_start(out=pool_s[:], in_=pool_ext[:])
            invc_s = cpool.tile([G, 1], F32)
            nc.sync.dma_start(out=invc_s[:], in_=invc_ext[:])
            fcw_s = cpool.tile([C, NCLS], F32)
            nc.sync.dma_start(out=fcw_s[:], in_=fcw_ext[:])
            fcb_s = cpool.tile([G, NCLS], F32)
            nc.sync.dma_start(out=fcb_s[:], in_=fcb_ext[:])
            es_all = cpool.tile([P, TOTCOL], I32)
            nc.sync.dma_start(out=es_all[:], in_=esrc_ext[:])
            ed_all = cpool.tile([P, TOTCOL], F32)
            nc.sync.dma_start(out=ed_all[:], in_=edst_ext[:])
            adst_a = cpool.tile([P, NT * H], F32)
            adst_b = cpool.tile([P, NT * H], F32)

            # ---- DRAM buffers ----
            hx_local = dpool.tile([NSHP, ROWW], F32)
            hx_fulls = [
                dpool.tile([NFULL, ROWW], F32, addr_space="Shared",
                           name=f"hx_full{i}")
                for i in range(3)
            ]
            pool_in = dpool.tile([G, C], F32)
            pool_out = dpool.tile([G, C], F32, addr_space="Shared")

            w_tiles = (w1_s, w2_s, w3_s)
            b_tiles = (b1_s, b2_s, b3_s)
            adst_of = (adst_a, adst_b, adst_a)

            def stage_a(xt, layer, t):
                """xt: SBUF [P, K] node-tile features for `layer`; emits
                [h | a_src] -> hx_local rows and a_dst -> adst_of[layer]."""
                K = F_IN if layer == 0 else HC
                nk = K // P
                w_s = w_tiles[layer]
                h_ps = pspool.tile([P, HC], F32, tag="big", bufs=3, name="h_ps")
                a_ps = pspool.tile([P, 2 * H], F32, tag="small", bufs=3,
                                   name="a_ps")
                for j in range(nk):
                    xT_ps = pspool.tile([P, P], F32, tag="trans", bufs=2,
                                        name="xT_ps")
                    nc.tensor.transpose(out=xT_ps[:], in_=xt[:, j * P:(j + 1) * P],
                                        identity=ident[:])
                    xT = wpool.tile([P, P], F32, tag="xT", name="xT")
                    nc.scalar.copy(xT[:], xT_ps[:])
                    nc.tensor.matmul(out=h_ps[:], lhsT=xT[:], rhs=w_s[:, j, 0:HC],
                                     start=(j == 0), stop=(j == nk - 1))
                    nc.tensor.matmul(out=a_ps[:], lhsT=xT[:],
                                     rhs=w_s[:, j, HC:WEXT],
                                     start=(j == 0), stop=(j == nk - 1))
                hx_t = wpool.tile([P, ROWW], F32, tag="hx_t", name="hx_t")
                nc.vector.tensor_copy(hx_t[:, 0:HC // 2], h_ps[:, 0:HC // 2])
                nc.scalar.copy(hx_t[:, HC // 2:HC], h_ps[:, HC // 2:HC])
                nc.vector.tensor_copy(hx_t[:, HC:ROWW], a_ps[:, 0:H])
                nc.vector.tensor_copy(
                    adst_of[layer][:, t * H:(t + 1) * H], a_ps[:, H:2 * H])
                nc.sync.dma_start(out=hx_local[t * P:(t + 1) * P, :], in_=hx_t[:])

            # ---- layer-0 stage A (from input features) ----
            for t in range(NT):
                xt = wpool.tile([P, F_IN], F32, tag="xt0", name="xt")
                nc.sync.dma_start(out=xt[:], in_=x_ext[t * P:(t + 1) * P, :])
                stage_a(xt, 0, t)
            nc.gpsimd.collective_compute(
                "AllGather", mybir.AluOpType.bypass, replica_groups=rg,
                ins=[hx_local[:]], outs=[hx_fulls[0][:]],
            )

            pool_ps = None
            for layer in range(3):
                hx_full = hx_fulls[layer]
                b_s = b_tiles[layer]
                adst_cur = adst_of[layer]
                if layer == 2:
                    pool_ps = pspool.tile([G, C], F32, tag="small", bufs=3,
                                          name="pool_ps")
                ch0 = 0
                for t in range(NT):
                    R_t, C_t = plan[t]
                    ncol = R_t + C_t
                    num_ps = pspool.tile([P, HC], F32, tag="big", bufs=3,
                                         name="num_ps")
                    den_ps = pspool.tile([P, H], F32, tag="small", bufs=3,
                                         name="den_ps")
                    a_d = adst_cur[:, t * H:(t + 1) * H]

                    for cc in range(ncol):
                        c = ch0 + cc
                        is_round = cc < R_t
                        first = cc == 0
                        last = cc == ncol - 1
                        gt = gpool.tile([P, ROWW], F32, tag="gt", name="gt")
                        nc.gpsimd.indirect_dma_start(
                            out=gt[:], out_offset=None,
                            in_=hx_full[:],
                            in_offset=bass.IndirectOffsetOnAxis(
                                ap=es_all[:, c:c + 1], axis=0),
                        )
                        ex = gpool.tile([P, H], F32, tag="ex", name="ex")
                        if is_round:
                            # dst == partition: direct adds, mask via bias
                            logit = gpool.tile([P, H], F32, tag="logit",
                                               name="logit")
                            nc.vector.tensor_add(logit[:], gt[:, HC:ROWW], a_d)
                            nc.vector.tensor_scalar_add(logit[:], logit[:],
                                                        ed_all[:, c:c + 1])
                            lr = gpool.tile([P, H], F32, tag="lr", name="lr")
                            nc.vector.tensor_scalar_mul(lr[:], logit[:], NEG)
                            nc.vector.tensor_tensor(out=lr[:], in0=lr[:],
                                                    in1=logit[:],
                                                    op=mybir.AluOpType.max)
                            nc.scalar.activation(ex[:], lr[:],
                                                 mybir.ActivationFunctionType.Exp)
                            lhs = ident
                        else:
                            oh = gpool.tile([P, P], F32, tag="oh", name="oh")
                            nc.vector.tensor_tensor(
                                out=oh[:],
                                in0=ed_all[:, c:c + 1].to_broadcast([P, P]),
                                in1=iota_f[:], op=mybir.AluOpType.is_equal)
                            ohT_ps = pspool.tile([P, P], F32, tag="trans",
                                                 bufs=2, name="ohT_ps")
                            nc.tensor.transpose(out=ohT_ps[:], in_=oh[:],
                                                identity=ident[:])
                            ohT = gpool.tile([P, P], F32, tag="ohT", name="ohT")
                            nc.scalar.copy(ohT[:], ohT_ps[:])
                            adpe_ps = pspool.tile([P, H], F32, tag="small",
                                                  bufs=3, name="adpe_ps")
                            nc.tensor.matmul(out=adpe_ps[:], lhsT=ohT[:],
                                             rhs=a_d, start=True, stop=True)
                            logit = gpool.tile([P, H], F32, tag="logit",
                                               name="logit")
                            nc.vector.tensor_add(logit[:], gt[:, HC:ROWW],
                                                 adpe_ps[:])
                            lr = gpool.tile([P, H], F32, tag="lr", name="lr")
                            nc.vector.tensor_scalar_mul(lr[:], logit[:], NEG)
                            nc.vector.tensor_tensor(out=lr[:], in0=lr[:],
                                                    in1=logit[:],
                                                    op=mybir.AluOpType.max)
                            nc.scalar.activation(ex[:], lr[:],
                                                 mybir.ActivationFunctionType.Exp)
                            lhs = oh
                        msg = gpool.tile([P, HC], F32, tag="msg", name="msg")
                        for h in range(H):
                            dst_sl = msg[:, h * C:(h + 1) * C]
                            src_sl = gt[:, h * C:(h + 1) * C]
                            if h % 2 == 0:
                                nc.vector.tensor_scalar_mul(dst_sl, src_sl,
                                                            ex[:, h:h + 1])
                            else:
                                nc.scalar.mul(dst_sl, src_sl, ex[:, h:h + 1])
                        nc.tensor.matmul(out=num_ps[:], lhsT=lhs[:], rhs=msg[:],
                                         start=first, stop=last)
                        nc.tensor.matmul(out=den_ps[:], lhsT=lhs[:], rhs=ex[:],
                                         start=first, stop=last)

                    den = wpool.tile([P, H], F32, tag="den", name="den")
                    nc.vector.tensor_scalar_add(den[:], den_ps[:], EPS)
                    rec = wpool.tile([P, H], F32, tag="rec", name="rec")
                    nc.vector.reciprocal(rec[:], den[:])
                    if layer < 2:
                        ot = wpool.tile([P, HC], F32, tag="ot", name="ot")
                        for h in range(H):
                            dst_sl = ot[:, h * C:(h + 1) * C]
                            src_sl = num_ps[:, h * C:(h + 1) * C]
                            if h % 2 == 0:
                                nc.vector.tensor_scalar_mul(dst_sl, src_sl,
                                                            rec[:, h:h + 1])
                            else:
                                nc.scalar.mul(dst_sl, src_sl, rec[:, h:h + 1])
                        ot2 = wpool.tile([P, HC], F32, tag="ot2", name="ot2")
                        nc.vector.tensor_add(ot2[:], ot[:], b_s[:])
                        ot3 = wpool.tile([P, HC], F32, tag="ot3", name="ot3")
                        nc.scalar.activation(ot3[:], ot2[:],
                                             mybir.ActivationFunctionType.Relu)
                        # fused next-layer stage A on this finished tile
                        stage_a(ot3, layer + 1, t)
                    else:
                        hm = wpool.tile([P, C], F32, tag="hm", name="hm")
                        nc.vector.tensor_scalar_mul(hm[:], num_ps[:, 0:C],
                                                    rec[:, 0:1])
                        for h in range(1, H):
                            hmt = wpool.tile([P, C], F32, tag="hmt", name="hmt")
                            if h % 2 == 0:
                                nc.vector.tensor_scalar_mul(
                                    hmt[:], num_ps[:, h * C:(h + 1) * C],
                                    rec[:, h:h + 1])
                            else:
                                nc.scalar.mul(hmt[:], num_ps[:, h * C:(h + 1) * C],
                                              rec[:, h:h + 1])
                            nc.vector.tensor_add(hm[:], hm[:], hmt[:])
                        hs = wpool.tile([P, C], F32, tag="hs", name="hs")
                        nc.vector.tensor_scalar_mul(hs[:], hm[:], 1.0 / H)
                        nc.vector.tensor_add(hs[:], hs[:], b3_s[:])
                        h3 = wpool.tile([P, C], F32, tag="h3", name="h3")
                        nc.scalar.activation(h3[:], hs[:],
                                             mybir.ActivationFunctionType.Relu)
                        poh = wpool.tile([P, G], F32, tag="poh", name="poh")
                        nc.vector.tensor_tensor(
                            out=poh[:], in0=pool_s[:, t:t + 1].to_broadcast([P, G]),
                            in1=iota_f[:, 0:G], op=mybir.AluOpType.is_equal)
                        nc.tensor.matmul(out=pool_ps[:], lhsT=poh[:], rhs=h3[:],
                                         start=(t == 0), stop=(t == NT - 1))
                    ch0 += ncol

                if layer < 2:
                    nc.gpsimd.collective_compute(
                        "AllGather", mybir.AluOpType.bypass, replica_groups=rg,
                        ins=[hx_local[:]], outs=[hx_fulls[layer + 1][:]],
                    )

            # ---- pool AllReduce + FC + log_softmax ----
            psb = wpool.tile([G, C], F32)
            nc.vector.tensor_copy(psb[:], pool_ps[:])
            nc.sync.dma_start(out=pool_in[:], in_=psb[:])
            nc.gpsimd.collective_compute(
                "AllReduce", mybir.AluOpType.add, replica_groups=rg,
                ins=[pool_in[:]], outs=[pool_out[:]],
            )
            pld = wpool.tile([G, C], F32)
            nc.sync.dma_start(out=pld[:], in_=pool_out[:])
            nc.vector.tensor_scalar_mul(pld[:], pld[:], invc_s[:, 0:1])
            pT_ps = pspool.tile([G, C], F32, tag="small", bufs=3, name="pT_ps")
            nc.tensor.transpose(out=pT_ps[:], in_=pld[:], identity=ident[0:G, 0:C])
            pT = wpool.tile([C, G], F32)
            nc.scalar.copy(pT[:], pT_ps[:])
            z_ps = pspool.tile([G, NCLS], F32, tag="small", bufs=3, name="z_ps")
            nc.tensor.matmul(out=z_ps[:], lhsT=pT[:], rhs=fcw_s[:],
                             start=True, stop=True)
            z = wpool.tile([G, NCLS], F32)
            nc.vector.tensor_add(z[:], z_ps[:], fcb_s[:])
            zm = wpool.tile([G, 1], F32)
            nc.vector.tensor_reduce(zm[:], z[:], axis=mybir.AxisListType.X,
                                    op=mybir.AluOpType.max)
            zs = wpool.tile([G, NCLS], F32)
            nc.vector.tensor_scalar_sub(zs[:], z[:], zm[:, 0:1])
            ze = wpool.tile([G, NCLS], F32)
            nc.scalar.activation(ze[:], zs[:], mybir.ActivationFunctionType.Exp)
            zsum = wpool.tile([G, 1], F32)
            nc.vector.tensor_reduce(zsum[:], ze[:], axis=mybir.AxisListType.X,
                                    op=mybir.AluOpType.add)
            zl = wpool.tile([G, 1], F32)
            nc.scalar.activation(zl[:], zsum[:], mybir.ActivationFunctionType.Ln)
            zo = wpool.tile([G, NCLS], F32)
            nc.vector.tensor_scalar_sub(zo[:], zs[:], zl[:, 0:1])
            nc.sync.dma_start(out=out_ext[:], in_=zo[:])

    nc.compile()
    return nc


def _prepare_inputs(x, edge_index, batch, W1, a1s, a1d, b1, W2, a2s, a2d, b2,
                    W3, a3s, a3d, b3, fcw, fcb):
    plan, esrcT, edstT = _preprocess_edges(np.asarray(edge_index, np.int64))
    x = np.asarray(x, np.float32)
    batch = np.asarray(batch, np.int64)

    shared = {
        "w1": _ext_weights(np.asarray(W1, np.float32), np.asarray(a1s, np.float32),
                           np.asarray(a1d, np.float32)),
        "w2": _ext_weights(np.asarray(W2, np.float32), np.asarray(a2s, np.float32),
                           np.asarray(a2d, np.float32)),
        "w3": _ext_weights(np.asarray(W3, np.float32), np.asarray(a3s, np.float32),
                           np.asarray(a3d, np.float32)),
        "b1r": np.tile(np.asarray(b1, np.float32)[None, :], (P, 1)),
        "b2r": np.tile(np.asarray(b2, np.float32)[None, :], (P, 1)),
        "b3r": np.tile(np.asarray(b3, np.float32)[None, :], (P, 1)),
        "fcw": np.asarray(fcw, np.float32),
        "fcbr": np.tile(np.asarray(fcb, np.float32)[None, :], (G, 1)),
        "invcnt": (1.0 / np.maximum(
            np.bincount(batch, minlength=G), 1.0)).astype(np.float32)[:, None],
    }

    in_maps = []
    for k in range(NCORES):
        xk = np.zeros((NSHP, F_IN), np.float32)
        xk[:NSH] = x[k * NSH:(k + 1) * NSH]
        pidx = np.full((NSHP,), PAD, np.float32)
        pidx[:NSH] = batch[k * NSH:(k + 1) * NSH]
        poolidx = np.ascontiguousarray(pidx.reshape(NT, P).T)  # [P, NT]
        in_maps.append({
            "x0": xk,
            "esrc": np.ascontiguousarray(esrcT[k]),
            "edst": np.ascontiguousarray(edstT[k]),
            "poolidx": poolidx,
            **shared,
        })
    return plan, in_maps


_CACHE = {}


def _get_nc(plan):
    key = tuple(plan)
    if key not in _CACHE:
        _CACHE[key] = _build_nc(plan)
    return _CACHE[key]


def kernel(x, edge_index, batch, W1, a1s, a1d, b1, W2, a2s, a2d, b2,
           W3, a3s, a3d, b3, fcw, fcb, _trace=False, _results=None):
    plan, in_maps = _prepare_inputs(x, edge_index, batch, W1, a1s, a1d, b1,
                                    W2, a2s, a2d, b2, W3, a3s, a3d, b3, fcw, fcb)
    nc = _get_nc(plan)
    res = run_bass_kernel_spmd(nc, in_maps, core_ids=list(range(NCORES)),
                               trace=_trace)
    if _results is not None:
        _results.append(res)
    return res.results[0]["out"]


# revision 21
# speedup vs baseline: 1435.9717x; 1052.3330x over previous
"""3-layer GAT (8 heads x 64 ch) + global mean pool + FC + log_softmax on 8 Trainium2 cores.

Sharding: nodes (and their incoming edges) are partitioned across the 8 cores;
GAT weights are replicated; per layer each core computes h=x@W for its node
shard, the [h | a_src] rows are AllGathered into a replicated table, and each
core gathers source rows for its edges with indirect DMA (one row per
partition per op — the HW limit).

Edge aggregation per 128-node dst tile is hybrid:
- "round" columns: round r holds each node's r-th incoming edge in that
  node's own partition, so dst == partition. a_dst adds directly, padding is
  a -1e30 bias before exp, and the weighted scatter-add is an
  identity-stationary matmul accumulating into PSUM.
- "overflow" columns: edges beyond the per-tile round cap go through a
  one-hot matmul path (is_equal against iota, PE transpose for the per-edge
  a_dst gather).
Numerator and softmax denominator accumulate in the same PSUM group; one
divide per tile. The next layer's x@W (stage A) is fused into the per-tile
finalize so PE work hides under the gather-DMA-bound phase. Graph mean-pool
partials are AllReduced at the end.
"""

import numpy as np

import concourse.bass as bass
import concourse.mybir as mybir
import concourse.tile as tile
from concourse import bacc
from concourse.bass_utils import run_bass_kernel_spmd
from concourse.masks import make_identity

# problem constants (hardcoded per contract)
N, E, F_IN, H, C, G, NCLS = 50000, 400000, 128, 8, 64, 64, 10
HC = H * C  # 512
NEG = 0.2
EPS = 1e-16

NCORES = 8
P = 128
NSH = N // NCORES          # 6250 nodes per core
NT = (NSH + P - 1) // P    # 49 dst tiles per core
NSHP = NT * P              # 6272 padded rows per core
NFULL = NCORES * NSHP      # 50176 rows in the gathered table
ROWW = HC + H              # 520: h | a_src
WEXT = HC + 2 * H          # 528: W | Wa_src | Wa_dst
PAD = 999.0                # one-hot miss marker for padded overflow slots
MASKNEG = -1.0e30          # pre-exp bias masking padded round slots
# AllGather split boundaries (tile granularity) for pipelining behind compute
SPLIT_TILES = (0, 49)
SPLIT_ROWS = tuple(t * P for t in SPLIT_TILES)

F32 = mybir.dt.float32
I32 = mybir.dt.int32


def _preprocess_edges(edge_index):
    """Assign edges (incl. self loops) to the dst-owning core; build per-tile
    round columns (node's r-th edge in its own partition) plus overflow
    one-hot chunk columns.

    Returns (plan, esrcT, edstT):
      plan: list of (R_t, C_t) per tile — shared by all cores.
      esrcT[k] int32 [P, TOTCOL]: gather row ids (pad -> 0).
      edstT[k] f32 [P, TOTCOL]: round cols -> 0.0 real / MASKNEG pad;
                                chunk cols -> dst slot or PAD."""
    src = np.concatenate([edge_index[0], np.arange(N, dtype=np.int64)])
    dst = np.concatenate([edge_index[1], np.arange(N, dtype=np.int64)])
    core = dst // NSH
    dloc = dst - core * NSH
    tile_of = dloc // P
    slot = dloc - tile_of * P
    # gather table layout: SPLITS segments, each rank-major over its row range
    sk = src // NSH
    sr = src % NSH
    split_rows = np.asarray(SPLIT_ROWS, np.int64)
    seg = np.searchsorted(split_rows[1:], sr, side="right")
    r0 = split_rows[seg]
    r1 = split_rows[seg + 1]
    gid = NCORES * r0 + sk * (r1 - r0) + (sr - r0)

    deg = np.zeros((NCORES, NT, P), np.int64)
    np.add.at(deg, (core, tile_of, slot), 1)
    maxdeg_t = deg.max(axis=(0, 2))  # [NT]

    # choose the round cap per tile: gathers dominate, one-hot chunk columns
    # carry ~25% extra compute
    R_ts = np.zeros(NT, np.int64)
    C_ts = np.zeros(NT, np.int64)
    for t in range(NT):
        best = None
        for rcap in range(1, int(maxdeg_t[t]) + 1):
            r = min(int(maxdeg_t[t]), rcap)
            ovf = np.maximum(deg[:, t, :] - rcap, 0).sum(axis=1)
            c = int(np.ceil(ovf / P).max())
            cost = r + 1.25 * c
            if best is None or cost < best[0]:
                best = (cost, r, c)
        _, R_ts[t], C_ts[t] = best
    plan = [(int(R_ts[t]), int(C_ts[t])) for t in range(NT)]
    colbase = np.zeros(NT, np.int64)
    colbase[1:] = np.cumsum(R_ts + C_ts)[:-1]
    TOTCOL = int((R_ts + C_ts).sum())

    esrcT = np.zeros((NCORES, P, TOTCOL), np.int32)
    edstT = np.empty((NCORES, P, TOTCOL), np.float32)
    for k in range(NCORES):
        # default fill: rounds masked, chunks PAD
        for t in range(NT):
            b = colbase[t]
            edstT[k, :, b:b + C_ts[t]] = PAD
            edstT[k, :, b + C_ts[t]:b + C_ts[t] + R_ts[t]] = MASKNEG
        m = core == k
        t_k, s_k, g_k = tile_of[m], slot[m], gid[m]
        order = np.argsort(t_k * P + s_k, kind="stable")
        t_k, s_k, g_k = t_k[order], s_k[order], g_k[order]
        node = t_k * P + s_k
        start = np.zeros(NT * P + 1, np.int64)
        np.add.at(start[1:], node, 1)
        start = np.cumsum(start)
        j = np.arange(len(node)) - start[node]  # rank within node
        rmax = R_ts[t_k]
        isr = j < rmax
        # round entries (after the C_t chunk columns)
        rcol = colbase[t_k[isr]] + C_ts[t_k[isr]] + j[isr]
        esrcT[k, s_k[isr], rcol] = g_k[isr].astype(np.int32)
        edstT[k, s_k[isr], rcol] = 0.0
        # overflow entries: sequential position within each tile
        to, so, go = t_k[~isr], s_k[~isr], g_k[~isr]
        oorder = np.argsort(to, kind="stable")
        to, so, go = to[oorder], so[oorder], go[oorder]
        ostart = np.zeros(NT + 1, np.int64)
        np.add.at(ostart[1:], to, 1)
        ostart = np.cumsum(ostart)
        q = np.arange(len(to)) - ostart[to]
        col = colbase[to] + q // P
        row = q % P
        esrcT[k, row, col] = go.astype(np.int32)
        edstT[k, row, col] = so.astype(np.float32)
    return plan, esrcT, edstT


def _ext_weights(W, a_s, a_d):
    """[K, 528] = [W | W@A_s | W@A_d] so h, a_src, a_dst come from one matmul."""
    K = W.shape[0]
    Wr = W.reshape(K, H, C)
    ws = np.einsum("fhc,hc->fh", Wr, a_s)
    wd = np.einsum("fhc,hc->fh", Wr, a_d)
    Wx = np.concatenate([W, ws, wd], axis=1).astype(np.float32)
    nk = K // P
    return np.ascontiguousarray(Wx.reshape(nk, P, WEXT).transpose(1, 0, 2))


def _build_nc(plan):
    TOTCOL = sum(r + c for r, c in plan)
    nc = bacc.Bacc("TRN2", target_bir_lowering=False, debug=False,
                   num_devices=NCORES)

    x_ext = nc.dram_tensor("x0", [NSHP, F_IN], F32, kind="ExternalInput")
    esrc_ext = nc.dram_tensor("esrc", [P, TOTCOL], I32, kind="ExternalInput")
    edst_ext = nc.dram_tensor("edst", [P, TOTCOL], F32, kind="ExternalInput")
    w1_ext = nc.dram_tensor("w1", [P, 1, WEXT], F32, kind="ExternalInput")
    w2_ext = nc.dram_tensor("w2", [P, 4, WEXT], F32, kind="ExternalInput")
    w3_ext = nc.dram_tensor("w3", [P, 4, WEXT], F32, kind="ExternalInput")
    b1_ext = nc.dram_tensor("b1r", [P, HC], F32, kind="ExternalInput")
    b2_ext = nc.dram_tensor("b2r", [P, HC], F32, kind="ExternalInput")
    b3_ext = nc.dram_tensor("b3r", [P, C], F32, kind="ExternalInput")
    pool_ext = nc.dram_tensor("poolidx", [P, NT], F32, kind="ExternalInput")
    invc_ext = nc.dram_tensor("invcnt", [G, 1], F32, kind="ExternalInput")
    fcw_ext = nc.dram_tensor("fcw", [C, NCLS], F32, kind="ExternalInput")
    fcb_ext = nc.dram_tensor("fcbr", [G, NCLS], F32, kind="ExternalInput")
    out_ext = nc.dram_tensor("out", [G, NCLS], F32, kind="ExternalOutput")

    rg = [list(range(NCORES))]

    with tile.TileContext(nc) as tc:
        with (
            tc.tile_pool(name="const", bufs=1) as cpool,
            tc.tile_pool(name="work", bufs=3) as wpool,
            tc.tile_pool(name="gat", bufs=10) as gpool,
            tc.tile_pool(name="ps", bufs=1, space="PSUM") as pspool,
            tc.tile_pool(name="dram", bufs=1, space="DRAM") as dpool,
        ):
            # ---- constants ----
            iota_i = cpool.tile([P, P], I32)
            nc.gpsimd.iota(iota_i[:], pattern=[[1, P]], base=0, channel_multiplier=0)
            iota_f = cpool.tile([P, P], F32)
            nc.vector.tensor_copy(iota_f[:], iota_i[:])
            ident = cpool.tile([P, P], F32)
            make_identity(nc, ident[:])

            w1_s = cpool.tile([P, 1, WEXT], F32)
            nc.sync.dma_start(out=w1_s[:], in_=w1_ext[:])
            w2_s = cpool.tile([P, 4, WEXT], F32)
            nc.sync.dma_start(out=w2_s[:], in_=w2_ext[:])
            w3_s = cpool.tile([P, 4, WEXT], F32)
            nc.sync.dma_start(out=w3_s[:], in_=w3_ext[:])
            b1_s = cpool.tile([P, HC], F32)
            nc.sync.dma_start(out=b1_s[:], in_=b1_ext[:])
            b2_s = cpool.tile([P, HC], F32)
            nc.sync.dma_start(out=b2_s[:], in_=b2_ext[:])
            b3_s = cpool.tile([P, C], F32)
            nc.sync.dma_start(out=b3_s[:], in_=b3_ext[:])
            pool_s = cpool.tile([P, NT], F32)
            nc.sync.dma_start(out=pool_s[:], in_=pool_ext[:])
            invc_s = cpool.tile([G, 1], F32)
            nc.sync.dma_start(out=invc_s[:], in_=invc_ext[:])
            fcw_s = cpool.tile([C, NCLS], F32)
            nc.sync.dma_start(out=fcw_s[:], in_=fcw_ext[:])
            fcb_s = cpool.tile([G, NCLS], F32)
            nc.sync.dma_start(out=fcb_s[:], in_=fcb_ext[:])
            es_all = cpool.tile([P, TOTCOL], I32)
            nc.sync.dma_start(out=es_all[:], in_=esrc_ext[:])
            ed_all = cpool.tile([P, TOTCOL], F32)
            nc.sync.dma_start(out=ed_all[:], in_=edst_ext[:])
            adst_a = cpool.tile([P, NT * H], F32)
            adst_b = cpool.tile([P, NT * H], F32)

            # ---- DRAM buffers ----
            hx_local = dpool.tile([NSHP, ROWW], F32)
            hx_fulls = [
                dpool.tile([NFULL, ROWW], F32, addr_space="Shared",
                           name=f"hx_full{i}")
                for i in range(3)
            ]
            pool_in = dpool.tile([G, C], F32)
            pool_out = dpool.tile([G, C], F32, addr_space="Shared")

            w_tiles = (w1_s, w2_s, w3_s)
            b_tiles = (b1_s, b2_s, b3_s)
            adst_of = (adst_a, adst_b, adst_a)
            split_end = {SPLIT_TILES[i + 1] - 1: i
                         for i in range(len(SPLIT_TILES) - 1)}

            def emit_split_ag(layer, seg):
                r0, r1 = SPLIT_ROWS[seg], SPLIT_ROWS[seg + 1]
                go = NCORES * r0
                nc.gpsimd.collective_compute(
                    "AllGather", mybir.AluOpType.bypass, replica_groups=rg,
                    ins=[hx_local[r0:r1, :]],
                    outs=[hx_fulls[layer][go:go + NCORES * (r1 - r0), :]],
                )

            def stage_a(xt, layer, t):
                """xt: SBUF [P, K] node-tile features for `layer`; emits
                [h | a_src] -> hx_local rows and a_dst -> adst_of[layer]."""
                K = F_IN if layer == 0 else HC
                nk = K // P
                w_s = w_tiles[layer]
                h_ps = pspool.tile([P, HC], F32, tag="big", bufs=3, name="h_ps")
                a_ps = pspool.tile([P, 2 * H], F32, tag="small", bufs=3,
                                   name="a_ps")
                for j in range(nk):
                    xT_ps = pspool.tile([P, P], F32, tag="trans", bufs=2,
                                        name="xT_ps")
                    nc.tensor.transpose(out=xT_ps[:], in_=xt[:, j * P:(j + 1) * P],
                                        identity=ident[:])
                    xT = wpool.tile([P, P], F32, tag="xT", name="xT")
                    nc.scalar.copy(xT[:], xT_ps[:])
                    nc.tensor.matmul(out=h_ps[:], lhsT=xT[:], rhs=w_s[:, j, 0:HC],
                                     start=(j == 0), stop=(j == nk - 1))
                    nc.tensor.matmul(out=a_ps[:], lhsT=xT[:],
                                     rhs=w_s[:, j, HC:WEXT],
                                     start=(j == 0), stop=(j == nk - 1))
                hx_t = wpool.tile([P, ROWW], F32, tag="hx_t", name="hx_t")
                nc.vector.tensor_copy(hx_t[:, 0:HC // 2], h_ps[:, 0:HC // 2])
                nc.scalar.copy(hx_t[:, HC // 2:HC], h_ps[:, HC // 2:HC])
                nc.vector.tensor_copy(hx_t[:, HC:ROWW], a_ps[:, 0:H])
                nc.vector.tensor_copy(
                    adst_of[layer][:, t * H:(t + 1) * H], a_ps[:, H:2 * H])
                nc.sync.dma_start(out=hx_local[t * P:(t + 1) * P, :], in_=hx_t[:])

            # ---- layer-0 stage A (from input features) ----
            for t in range(NT):
                xt = wpool.tile([P, F_IN], F32, tag="xt0", name="xt")
                nc.sync.dma_start(out=xt[:], in_=x_ext[t * P:(t + 1) * P, :])
                stage_a(xt, 0, t)
                if t in split_end:
                    emit_split_ag(0, split_end[t])

            pool_ps = None
            for layer in range(3):
                hx_full = hx_fulls[layer]
                b_s = b_tiles[layer]
                adst_cur = adst_of[layer]
                if layer == 2:
                    pool_ps = pspool.tile([G, C], F32, tag="small", bufs=3,
                                          name="pool_ps")
                ch0 = 0
                for t in range(NT):
                    R_t, C_t = plan[t]
                    ncol = R_t + C_t
                    num_ps = pspool.tile([P, HC], F32, tag="big", bufs=3,
                                         name="num_ps")
                    den_acc = wpool.tile([P, H], F32, tag="den_acc",
                                         name="den_acc")
                    den_ps = None
                    if C_t > 0:
                        den_ps = pspool.tile([P, H], F32, tag="small", bufs=3,
                                             name="den_ps")
                    a_d = adst_cur[:, t * H:(t + 1) * H]

                    for cc in range(ncol):
                        c = ch0 + cc
                        is_round = cc >= C_t
                        first = cc == 0
                        last = cc == ncol - 1
                        gt = gpool.tile([P, ROWW], F32, tag="gt", name="gt")
                        nc.gpsimd.indirect_dma_start(
                            out=gt[:], out_offset=None,
                            in_=hx_full[:],
                            in_offset=bass.IndirectOffsetOnAxis(
                                ap=es_all[:, c:c + 1], axis=0),
                        )
                        ex = gpool.tile([P, H], F32, tag="ex", name="ex")
                        if is_round:
                            # dst == partition: direct adds, mask via bias
                            logit = gpool.tile([P, H], F32, tag="logit",
                                               name="logit")
                            nc.vector.tensor_add(logit[:], gt[:, HC:ROWW], a_d)
                            nc.vector.tensor_scalar_add(logit[:], logit[:],
                                                        ed_all[:, c:c + 1])
                            lr = gpool.tile([P, H], F32, tag="lr", name="lr")
                            nc.vector.tensor_scalar_mul(lr[:], logit[:], NEG)
                            nc.vector.tensor_tensor(out=lr[:], in0=lr[:],
                                                    in1=logit[:],
                                                    op=mybir.AluOpType.max)
                            nc.scalar.activation(ex[:], lr[:],
                                                 mybir.ActivationFunctionType.Exp)
                            lhs = ident
                        else:
                            oh = gpool.tile([P, P], F32, tag="oh", name="oh")
                            nc.vector.tensor_tensor(
                                out=oh[:],
                                in0=ed_all[:, c:c + 1].to_broadcast([P, P]),
                                in1=iota_f[:], op=mybir.AluOpType.is_equal)
                            ohT_ps = pspool.tile([P, P], F32, tag="trans",
                                                 bufs=2, name="ohT_ps")
                            nc.tensor.transpose(out=ohT_ps[:], in_=oh[:],
                                                identity=ident[:])
                            ohT = gpool.tile([P, P], F32, tag="ohT", name="ohT")
                            nc.scalar.copy(ohT[:], ohT_ps[:])
                            adpe_ps = pspool.tile([P, H], F32, tag="small",
                                                  bufs=3, name="adpe_ps")
                            nc.tensor.matmul(out=adpe_ps[:], lhsT=ohT[:],
                                             rhs=a_d, start=True, stop=True)
                            logit = gpool.tile([P, H], F32, tag="logit",
                                               name="logit")
                            nc.vector.tensor_add(logit[:], gt[:, HC:ROWW],
                                                 adpe_ps[:])
                            lr = gpool.tile([P, H], F32, tag="lr", name="lr")
                            nc.vector.tensor_scalar_mul(lr[:], logit[:], NEG)
                            nc.vector.tensor_tensor(out=lr[:], in0=lr[:],
                                                    in1=logit[:],
                                                    op=mybir.AluOpType.max)
                            nc.scalar.activation(ex[:], lr[:],
                                                 mybir.ActivationFunctionType.Exp)
                            lhs = oh
                        msg = gpool.tile([P, HC], F32, tag="msg", name="msg")
                        for h in range(H):
                            dst_sl = msg[:, h * C:(h + 1) * C]
                            src_sl = gt[:, h * C:(h + 1) * C]
                            if h % 2 == 0:
                                nc.vector.tensor_scalar_mul(dst_sl, src_sl,
                                                            ex[:, h:h + 1])
                            else:
                                nc.scalar.mul(dst_sl, src_sl, ex[:, h:h + 1])
                        nc.tensor.matmul(out=num_ps[:], lhsT=lhs[:], rhs=msg[:],
                                         start=first, stop=last)
                        if is_round:
                            if cc == C_t:
                                nc.vector.tensor_copy(den_acc[:], ex[:])
                            else:
                                nc.vector.tensor_add(den_acc[:], den_acc[:],
                                                     ex[:])
                        else:
                            nc.tensor.matmul(out=den_ps[:], lhsT=lhs[:],
                                             rhs=ex[:], start=(cc == 0),
                                             stop=(cc == C_t - 1))

                    den = wpool.tile([P, H], F32, tag="den", name="den")
                    if den_ps is not None:
                        nc.vector.tensor_add(den[:], den_acc[:], den_ps[:])
                        nc.vector.tensor_scalar_add(den[:], den[:], EPS)
                    else:
                        nc.vector.tensor_scalar_add(den[:], den_acc[:], EPS)
                    rec = wpool.tile([P, H], F32, tag="rec", name="rec")
                    nc.vector.reciprocal(rec[:], den[:])
                    if layer < 2:
                        ot = wpool.tile([P, HC], F32, tag="ot", name="ot")
                        for h in range(H):
                            dst_sl = ot[:, h * C:(h + 1) * C]
                            src_sl = num_ps[:, h * C:(h + 1) * C]
                            if h % 2 == 0:
                                nc.vector.tensor_scalar_mul(dst_sl, src_sl,
                                                            rec[:, h:h + 1])
                            else:
                                nc.scalar.mul(dst_sl, src_sl, rec[:, h:h + 1])
                        ot2 = wpool.tile([P, HC], F32, tag="ot2", name="ot2")
                        nc.vector.tensor_add(ot2[:], ot[:], b_s[:])
                        ot3 = wpool.tile([P, HC], F32, tag="ot3", name="ot3")
                        nc.scalar.activation(ot3[:], ot2[:],
                                             mybir.ActivationFunctionType.Relu)
                        # fused next-layer stage A on this finished tile
                        stage_a(ot3, layer + 1, t)
                        if t in split_end:
                            emit_split_ag(layer + 1, split_end[t])
                    else:
                        hm = wpool.tile([P, C], F32, tag="hm", name="hm")
                        nc.vector.tensor_scalar_mul(hm[:], num_ps[:, 0:C],
                                                    rec[:, 0:1])
                        for h in range(1, H):
                            hmt = wpool.tile([P, C], F32, tag="hmt", name="hmt")
                            if h % 2 == 0:
                                nc.vector.tensor_scalar_mul(
                                    hmt[:], num_ps[:, h * C:(h + 1) * C],
                                    rec[:, h:h + 1])
                            else:
                                nc.scalar.mul(hmt[:], num_ps[:, h * C:(h + 1) * C],
                                              rec[:, h:h + 1])
                            nc.vector.tensor_add(hm[:], hm[:], hmt[:])
                        hs = wpool.tile([P, C], F32, tag="hs", name="hs")
                        nc.vector.tensor_scalar_mul(hs[:], hm[:], 1.0 / H)
                        nc.vector.tensor_add(hs[:], hs[:], b3_s[:])
                        h3 = wpool.tile([P, C], F32, tag="h3", name="h3")
                        nc.scalar.activation(h3[:], hs[:],
                                             mybir.ActivationFunctionType.Relu)
                        poh = wpool.tile([P, G], F32, tag="poh", name="poh")
                        nc.vector.tensor_tensor(
                            out=poh[:], in0=pool_s[:, t:t + 1].to_broadcast([P, G]),
                            in1=iota_f[:, 0:G], op=mybir.AluOpType.is_equal)
                        nc.tensor.matmul(out=pool_ps[:], lhsT=poh[:], rhs=h3[:],
                                         start=(t == 0), stop=(t == NT - 1))
                    ch0 += ncol

            # ---- pool AllReduce + FC + log_softmax ----
            psb = wpool.tile([G, C], F32)
            nc.vector.tensor_copy(psb[:], pool_ps[:])
            nc.sync.dma_start(out=pool_in[:], in_=psb[:])
            nc.gpsimd.collective_compute(
                "AllReduce", mybir.AluOpType.add, replica_groups=rg,
                ins=[pool_in[:]], outs=[pool_out[:]],
            )
            pld = wpool.tile([G, C], F32)
            nc.sync.dma_start(out=pld[:], in_=pool_out[:])
            nc.vector.tensor_scalar_mul(pld[:], pld[:], invc_s[:, 0:1])
            pT_ps = pspool.tile([G, C], F32, tag="small", bufs=3, name="pT_ps")
            nc.tensor.transpose(out=pT_ps[:], in_=pld[:], identity=ident[0:G, 0:C])
            pT = wpool.tile([C, G], F32)
            nc.scalar.copy(pT[:], pT_ps[:])
            z_ps = pspool.tile([G, NCLS], F32, tag="small", bufs=3, name="z_ps")
            nc.tensor.matmul(out=z_ps[:], lhsT=pT[:], rhs=fcw_s[:],
                             start=True, stop=True)
            z = wpool.tile([G, NCLS], F32)
            nc.vector.tensor_add(z[:], z_ps[:], fcb_s[:])
            zm = wpool.tile([G, 1], F32)
            nc.vector.tensor_reduce(zm[:], z[:], axis=mybir.AxisListType.X,
                                    op=mybir.AluOpType.max)
            zs = wpool.tile([G, NCLS], F32)
            nc.vector.tensor_scalar_sub(zs[:], z[:], zm[:, 0:1])
            ze = wpool.tile([G, NCLS], F32)
            nc.scalar.activation(ze[:], zs[:], mybir.ActivationFunctionType.Exp)
            zsum = wpool.tile([G, 1], F32)
            nc.vector.tensor_reduce(zsum[:], ze[:], axis=mybir.AxisListType.X,
                                    op=mybir.AluOpType.add)
            zl = wpool.tile([G, 1], F32)
            nc.scalar.activation(zl[:], zsum[:], mybir.ActivationFunctionType.Ln)
            zo = wpool.tile([G, NCLS], F32)
            nc.vector.tensor_scalar_sub(zo[:], zs[:], zl[:, 0:1])
            nc.sync.dma_start(out=out_ext[:], in_=zo[:])

    nc.compile()
    return nc


def _prepare_inputs(x, edge_index, batch, W1, a1s, a1d, b1, W2, a2s, a2d, b2,
                    W3, a3s, a3d, b3, fcw, fcb):
    plan, esrcT, edstT = _preprocess_edges(np.asarray(edge_index, np.int64))
    x = np.asarray(x, np.float32)
    batch = np.asarray(batch, np.int64)

    shared = {
        "w1": _ext_weights(np.asarray(W1, np.float32), np.asarray(a1s, np.float32),
                           np.asarray(a1d, np.float32)),
        "w2": _ext_weights(np.asarray(W2, np.float32), np.asarray(a2s, np.float32),
                           np.asarray(a2d, np.float32)),
        "w3": _ext_weights(np.asarray(W3, np.float32), np.asarray(a3s, np.float32),
                           np.asarray(a3d, np.float32)),
        "b1r": np.tile(np.asarray(b1, np.float32)[None, :], (P, 1)),
        "b2r": np.tile(np.asarray(b2, np.float32)[None, :], (P, 1)),
        "b3r": np.tile(np.asarray(b3, np.float32)[None, :], (P, 1)),
        "fcw": np.asarray(fcw, np.float32),
        "fcbr": np.tile(np.asarray(fcb, np.float32)[None, :], (G, 1)),
        "invcnt": (1.0 / np.maximum(
            np.bincount(batch, minlength=G), 1.0)).astype(np.float32)[:, None],
    }

    in_maps = []
    for k in range(NCORES):
        xk = np.zeros((NSHP, F_IN), np.float32)
        xk[:NSH] = x[k * NSH:(k + 1) * NSH]
        pidx = np.full((NSHP,), PAD, np.float32)
        pidx[:NSH] = batch[k * NSH:(k + 1) * NSH]
        poolidx = np.ascontiguousarray(pidx.reshape(NT, P).T)  # [P, NT]
        in_maps.append({
            "x0": xk,
            "esrc": np.ascontiguousarray(esrcT[k]),
            "edst": np.ascontiguousarray(edstT[k]),
            "poolidx": poolidx,
            **shared,
        })
    return plan, in_maps


_CACHE = {}


def _get_nc(plan):
    key = tuple(plan)
    if key not in _CACHE:
        _CACHE[key] = _build_nc(plan)
    return _CACHE[key]


def kernel(x, edge_index, batch, W1, a1s, a1d, b1, W2, a2s, a2d, b2,
           W3, a3s, a3d, b3, fcw, fcb, _trace=False, _results=None):
    plan, in_maps = _prepare_inputs(x, edge_index, batch, W1, a1s, a1d, b1,
                                    W2, a2s, a2d, b2, W3, a3s, a3d, b3, fcw, fcb)
    nc = _get_nc(plan)
    res = run_bass_kernel_spmd(nc, in_maps, core_ids=list(range(NCORES)),
                               trace=_trace)
    if _results is not None:
        _results.append(res)
    return res.results[0]["out"]
